# revision 1
# baseline (speedup 1.0000x reference)
"""Tensor-parallel SwiGLU MLP (LLaMA-style) on 8 Trainium2 NeuronCores.

Problem: y = (silu(x @ Wg^T) * (x @ Wu^T)) @ Wd^T
  x [2, 2048, 4096] f32, Wg/Wu [11008, 4096] f32, Wd [4096, 11008] f32.

Sharding (tensor-parallel over d_ff): each core gets a 1376-wide slice of
the intermediate dimension (zero-padded to 1408 = 11*128), computes its
partial y^T in transposed layout, and the host sums the 8 partials.

Compute is bf16 on the TensorEngine with f32 PSUM accumulation. All DRAM
tensors are pre-laid-out on the host so that every DMA is partition-major
contiguous. Weight loads ride the sync-engine HWDGE queue, x loads the
gpsimd SWDGE queue, outputs the scalar-engine HWDGE queue, so the big x
prefetch never head-of-line-blocks weight tiles. A block of dummy warmup
matmuls trips the PE HAM clock-gate to 2.4 GHz while the first DMAs land.

kernel(**inputs) -> np.ndarray [2, 2048, 4096] f32.
Set env MLP_KERNEL_TRACE=1 to capture a neuron-profile; the measured
exec_time_ns is then stored in LAST_EXEC_TIME_NS.
"""

import os
import sys
import types

import numpy as np
import ml_dtypes

import concourse.bacc as bacc
import concourse.mybir as mybir
import concourse.tile as tile
from concourse.bass_utils import run_bass_kernel_spmd

P = 128
D = 4096            # d_model
DFF = 11008
NCORES = 8
F = DFF // NCORES   # 1376 per core
FP = 1408           # padded to a multiple of 128 (zero rows/cols)
T = 4096            # tokens (2 * 2048)
KD = D // P         # 32 k-subtiles for gate/up
NF = FP // P        # 11 f-chunks (k-subtiles for the down proj)
MD = D // P         # 32 output row chunks

TT = 1024           # tokens per pass
NFREE = 512         # matmul moving-dim / PSUM bank size (f32)
NP = T // TT
NI = TT // NFREE
XCH = 8             # x DMA chunks per pass
WARMUP = 120

BF16 = mybir.dt.bfloat16
F32 = mybir.dt.float32
NPBF16 = ml_dtypes.bfloat16

LAST_EXEC_TIME_NS = None
_CACHED_NC = None


def _build():
    nc = bacc.Bacc("TRN2", target_bir_lowering=False, debug=False)

    xh = nc.dram_tensor("xh", [NP, P, KD, TT], BF16, kind="ExternalInput")
    wg = nc.dram_tensor("wg", [NF, P, KD, P], BF16, kind="ExternalInput")
    wu = nc.dram_tensor("wu", [NF, P, KD, P], BF16, kind="ExternalInput")
    wd = nc.dram_tensor("wd", [MD, P, NF, P], BF16, kind="ExternalInput")
    y = nc.dram_tensor("y", [MD, P, T], F32, kind="ExternalOutput")

    silu = mybir.ActivationFunctionType.Silu

    with tile.TileContext(nc) as tc:
        with (
            tc.tile_pool(name="xp", bufs=1) as xp,
            tc.tile_pool(name="wgp", bufs=3) as wgp,
            tc.tile_pool(name="wup", bufs=3) as wup,
            tc.tile_pool(name="wdp", bufs=6) as wdp,
            tc.tile_pool(name="hp", bufs=2) as hp,
            tc.tile_pool(name="gp", bufs=4) as gp,
            tc.tile_pool(name="op", bufs=4) as op,
            tc.tile_pool(name="ps", bufs=2, space="PSUM") as ps,
        ):
            # Warm the PE HAM clock-gate while the first DMAs are in
            # flight: dummy matmuls on a zeroed scratch tile into a
            # scratch PSUM bank nobody reads.
            wsc = gp.tile([P, 2 * P], BF16, name="wsc", tag="wsc", bufs=1)
            nc.vector.memset(wsc[:], 0.0)
            pw = ps.tile([P, P], F32, name="pw", tag="pw", bufs=1)
            for _ in range(WARMUP):
                nc.tensor.matmul(pw[:], wsc[:, :P], wsc[:, P:],
                                 start=True, stop=True)

            KCH = KD // XCH
            for tp in range(NP):
                xt = xp.tile([P, KD, TT], BF16, name="xt", tag="xt")
                # x chunked so the first k-subtiles land early; pass 0's
                # first chunks ride the (empty, low-latency) sync queue
                for c in range(XCH):
                    ks = slice(c * KCH, (c + 1) * KCH)
                    eng = nc.sync if (tp == 0 and c < 2) else nc.gpsimd
                    eng.dma_start(xt[:, ks, :], xh[tp, :, ks, :])
                ht = hp.tile([P, NF, TT], BF16, name="ht", tag="ht")
                for fi in range(NF):
                    wgt = wgp.tile([P, KD, P], BF16, name="wgt", tag="wgt")
                    nc.sync.dma_start(wgt[:], wg[fi])
                    wut = wup.tile([P, KD, P], BF16, name="wut", tag="wut")
                    nc.sync.dma_start(wut[:], wu[fi])
                    for ni in range(NI):
                        sl = slice(ni * NFREE, (ni + 1) * NFREE)
                        pg = ps.tile([P, NFREE], F32, name="pg", tag="pg")
                        for k in range(KD):
                            nc.tensor.matmul(pg[:], wgt[:, k, :], xt[:, k, sl],
                                             start=(k == 0), stop=(k == KD - 1))
                        pu = ps.tile([P, NFREE], F32, name="pu", tag="pu")
                        for k in range(KD):
                            nc.tensor.matmul(pu[:], wut[:, k, :], xt[:, k, sl],
                                             start=(k == 0), stop=(k == KD - 1))
                        gt = gp.tile([P, NFREE], BF16, name="gt", tag="gt")
                        nc.scalar.activation(gt[:], pg[:], silu)
                        nc.vector.tensor_mul(ht[:, fi, sl], pu[:], gt[:])
                for mi in range(MD):
                    wdt = wdp.tile([P, NF, P], BF16, name="wdt", tag="wdt")
                    nc.sync.dma_start(wdt[:], wd[mi])
                    for ni in range(NI):
                        sl = slice(ni * NFREE, (ni + 1) * NFREE)
                        py = ps.tile([P, NFREE], F32, name="py", tag="py",
                                     bufs=3)
                        for k in range(NF):
                            nc.tensor.matmul(py[:], wdt[:, k, :], ht[:, k, sl],
                                             start=(k == 0), stop=(k == NF - 1))
                        ot = op.tile([P, NFREE], F32, name="ot", tag="ot")
                        # evict on ACT so DVE only produces h (the last h
                        # chunks must never queue behind y copies)
                        nc.scalar.copy(ot[:], py[:])
                        off = tp * TT + ni * NFREE
                        # outputs on the scalar-engine HWDGE queue so they
                        # never block weight loads on the sync queue
                        nc.scalar.dma_start(y[mi, :, off:off + NFREE], ot[:])

    nc.compile()
    return nc


def _prep_inputs(x, W_gate, W_up, W_down):
    xf = np.ascontiguousarray(np.asarray(x, dtype=np.float32)).reshape(T, D)
    # xh[tp, p, k, t] = x[tp*TT + t, k*128 + p]
    xh = np.ascontiguousarray(
        xf.reshape(NP, TT, KD, P).transpose(0, 3, 2, 1)).astype(NPBF16)

    Wg = np.asarray(W_gate, dtype=np.float32)
    Wu = np.asarray(W_up, dtype=np.float32)
    Wd = np.asarray(W_down, dtype=np.float32)

    in_maps = []
    pad_r = np.zeros((FP - F, D), np.float32)
    pad_c = np.zeros((D, FP - F), np.float32)
    for c in range(NCORES):
        fs = c * F
        # wg[fi, p, k, j] = Wg_pad[fi*128 + j, k*128 + p]
        wgs = np.ascontiguousarray(
            np.concatenate([Wg[fs:fs + F], pad_r], axis=0)
            .reshape(NF, P, KD, P).transpose(0, 3, 2, 1)).astype(NPBF16)
        wus = np.ascontiguousarray(
            np.concatenate([Wu[fs:fs + F], pad_r], axis=0)
            .reshape(NF, P, KD, P).transpose(0, 3, 2, 1)).astype(NPBF16)
        # wd[mi, p, k, j] = Wd_pad[mi*128 + j, fs + k*128 + p]
        wds = np.ascontiguousarray(
            np.concatenate([Wd[:, fs:fs + F], pad_c], axis=1)
            .reshape(MD, P, NF, P).transpose(0, 3, 2, 1)).astype(NPBF16)
        in_maps.append({"xh": xh, "wg": wgs, "wu": wus, "wd": wds})
    return in_maps


def _install_ntff_shim():
    """antenv.axon_hooks is missing from some images; register an
    equivalent module so trace=True can capture NTFF profiles."""
    try:
        import antenv.axon_hooks  # noqa: F401
        return True
    except ImportError:
        pass
    try:
        import antenv
        from trn_agent_boot.trn_boot import _ntff_profile_via_ctypes
        hook = _ntff_profile_via_ctypes('/opt/axon/libaxon_pjrt.so')
        mod = types.ModuleType("antenv.axon_hooks")
        mod._hook = hook
        mod.get_axon_ntff_profile_hook = lambda: mod._hook

        def set_axon_ntff_profile_hook(h):
            mod._hook = h

        mod.set_axon_ntff_profile_hook = set_axon_ntff_profile_hook
        sys.modules["antenv.axon_hooks"] = mod
        antenv.axon_hooks = mod
        return True
    except Exception:
        return False


def kernel(x, W_gate, W_up, W_down):
    global LAST_EXEC_TIME_NS, _CACHED_NC
    if _CACHED_NC is None:
        _CACHED_NC = _build()
    nc = _CACHED_NC

    in_maps = _prep_inputs(x, W_gate, W_up, W_down)

    trace = os.environ.get("MLP_KERNEL_TRACE", "0") == "1"
    if trace:
        trace = _install_ntff_shim()

    res = run_bass_kernel_spmd(nc, in_maps, list(range(NCORES)), trace=trace)
    LAST_EXEC_TIME_NS = res.exec_time_ns

    # sum per-core partials: acc[mi, p, t] = y^T[mi*128+p, t]
    acc = res.results[0]["y"].astype(np.float32, copy=True)
    for r in res.results[1:]:
        acc += r["y"]
    yout = np.ascontiguousarray(acc.transpose(2, 0, 1).reshape(T, D))
    return yout.reshape(2, 2048, D)


# revision 4
# speedup vs baseline: 1.0037x; 1.0037x over previous
"""Tensor-parallel SwiGLU MLP (LLaMA-style) on 8 Trainium2 NeuronCores.

Problem: y = (silu(x @ Wg^T) * (x @ Wu^T)) @ Wd^T
  x [2, 2048, 4096] f32, Wg/Wu [11008, 4096] f32, Wd [4096, 11008] f32.

Sharding (tensor-parallel over d_ff): each core gets a 1376-wide slice of
the intermediate dimension (zero-padded to 1408 = 11*128), computes its
partial y^T in transposed layout, and the host sums the 8 partials.

Compute is bf16 on the TensorEngine with f32 PSUM accumulation. All DRAM
tensors are pre-laid-out on the host so that every DMA is partition-major
contiguous. Weight loads ride the sync-engine HWDGE queue, x loads the
gpsimd SWDGE queue, outputs the scalar-engine HWDGE queue, so the big x
prefetch never head-of-line-blocks weight tiles. A block of dummy warmup
matmuls trips the PE HAM clock-gate to 2.4 GHz while the first DMAs land.

kernel(**inputs) -> np.ndarray [2, 2048, 4096] f32.
Set env MLP_KERNEL_TRACE=1 to capture a neuron-profile; the measured
exec_time_ns is then stored in LAST_EXEC_TIME_NS.
"""

import os
import sys
import types

import numpy as np
import ml_dtypes

import concourse.bacc as bacc
import concourse.mybir as mybir
import concourse.tile as tile
from concourse.bass_utils import run_bass_kernel_spmd

P = 128
D = 4096            # d_model
DFF = 11008
NCORES = 8
F = DFF // NCORES   # 1376 per core
FP = 1408           # padded to a multiple of 128 (zero rows/cols)
T = 4096            # tokens (2 * 2048)
KD = D // P         # 32 k-subtiles for gate/up
NF = FP // P        # 11 f-chunks (k-subtiles for the down proj)
MD = D // P         # 32 output row chunks

TT = 1024           # tokens per pass
NFREE = 512         # matmul moving-dim / PSUM bank size (f32)
NP = T // TT
NI = TT // NFREE
XCH = 8             # x DMA chunks per pass
WARMUP = 300

BF16 = mybir.dt.bfloat16
F32 = mybir.dt.float32
NPBF16 = ml_dtypes.bfloat16

LAST_EXEC_TIME_NS = None
_CACHED_NC = None


def _build():
    nc = bacc.Bacc("TRN2", target_bir_lowering=False, debug=False)

    xh = nc.dram_tensor("xh", [NP, P, KD, TT], BF16, kind="ExternalInput")
    wg = nc.dram_tensor("wg", [NF, P, KD, P], BF16, kind="ExternalInput")
    wu = nc.dram_tensor("wu", [NF, P, KD, P], BF16, kind="ExternalInput")
    wd = nc.dram_tensor("wd", [MD, P, NF, P], BF16, kind="ExternalInput")
    y = nc.dram_tensor("y", [MD, P, T], F32, kind="ExternalOutput")

    silu = mybir.ActivationFunctionType.Silu

    with tile.TileContext(nc) as tc:
        with (
            tc.tile_pool(name="xp", bufs=1) as xp,
            tc.tile_pool(name="wgp", bufs=3) as wgp,
            tc.tile_pool(name="wup", bufs=3) as wup,
            tc.tile_pool(name="wdp", bufs=6) as wdp,
            tc.tile_pool(name="hp", bufs=2) as hp,
            tc.tile_pool(name="gp", bufs=4) as gp,
            tc.tile_pool(name="op", bufs=4) as op,
            tc.tile_pool(name="ps", bufs=2, space="PSUM") as ps,
        ):
            # Warm the PE HAM clock-gate while the first DMAs are in
            # flight: dummy matmuls on a zeroed scratch tile into a
            # scratch PSUM bank nobody reads (shares the pg tag's banks).
            wsc = gp.tile([P, 2 * P], BF16, name="wsc", tag="wsc", bufs=1)
            nc.vector.memset(wsc[:], 0.0)
            pw = ps.tile([P, P], F32, name="pw", tag="pg")
            for _ in range(WARMUP):
                nc.tensor.matmul(pw[:], wsc[:, :P], wsc[:, P:],
                                 start=True, stop=True)

            KCH = KD // XCH
            for tp in range(NP):
                xt = xp.tile([P, KD, TT], BF16, name="xt", tag="xt")
                # x chunked so the first k-subtiles land early; pass 0's
                # first chunks ride the (empty, low-latency) sync queue
                for c in range(XCH):
                    ks = slice(c * KCH, (c + 1) * KCH)
                    eng = nc.sync if (tp == 0 and c < 2) else nc.gpsimd
                    eng.dma_start(xt[:, ks, :], xh[tp, :, ks, :])
                ht = hp.tile([P, NF, TT], BF16, name="ht", tag="ht")
                for fi in range(NF):
                    wgt = wgp.tile([P, KD, P], BF16, name="wgt", tag="wgt")
                    nc.sync.dma_start(wgt[:], wg[fi])
                    wut = wup.tile([P, KD, P], BF16, name="wut", tag="wut")
                    nc.sync.dma_start(wut[:], wu[fi])
                    for ni in range(NI):
                        sl = slice(ni * NFREE, (ni + 1) * NFREE)
                        pg = ps.tile([P, NFREE], F32, name="pg", tag="pg")
                        for k in range(KD):
                            nc.tensor.matmul(pg[:], wgt[:, k, :], xt[:, k, sl],
                                             start=(k == 0), stop=(k == KD - 1))
                        pu = ps.tile([P, NFREE], F32, name="pu", tag="pu")
                        for k in range(KD):
                            nc.tensor.matmul(pu[:], wut[:, k, :], xt[:, k, sl],
                                             start=(k == 0), stop=(k == KD - 1))
                        gt = gp.tile([P, NFREE], BF16, name="gt", tag="gt")
                        nc.scalar.activation(gt[:], pg[:], silu)
                        nc.vector.tensor_mul(ht[:, fi, sl], pu[:], gt[:])
                for mi in range(MD):
                    wdt = wdp.tile([P, NF, P], BF16, name="wdt", tag="wdt")
                    nc.sync.dma_start(wdt[:], wd[mi])
                    for ni in range(NI):
                        sl = slice(ni * NFREE, (ni + 1) * NFREE)
                        py = ps.tile([P, NFREE], F32, name="py", tag="py",
                                     bufs=4)
                        for k in range(NF):
                            nc.tensor.matmul(py[:], wdt[:, k, :], ht[:, k, sl],
                                             start=(k == 0), stop=(k == NF - 1))
                        ot = op.tile([P, NFREE], F32, name="ot", tag="ot")
                        # Evictions alternate ACT/DVE and are emitted at
                        # high scheduler priority: a single eviction FIFO
                        # gets head-of-line blocked behind silu ops that
                        # wait on the PE, which in turn waits on the PSUM
                        # bank the eviction would free.
                        with tc.high_priority():
                            if ni % 2 == 0:
                                nc.scalar.copy(ot[:], py[:])
                            else:
                                nc.vector.tensor_copy(ot[:], py[:])
                            off = tp * TT + ni * NFREE
                            # outputs on the scalar-engine HWDGE queue so
                            # they never block weight loads on the sync queue
                            nc.scalar.dma_start(y[mi, :, off:off + NFREE],
                                                ot[:])

    nc.compile()
    return nc


def _prep_inputs(x, W_gate, W_up, W_down):
    xf = np.ascontiguousarray(np.asarray(x, dtype=np.float32)).reshape(T, D)
    # xh[tp, p, k, t] = x[tp*TT + t, k*128 + p]
    xh = np.ascontiguousarray(
        xf.reshape(NP, TT, KD, P).transpose(0, 3, 2, 1)).astype(NPBF16)

    Wg = np.asarray(W_gate, dtype=np.float32)
    Wu = np.asarray(W_up, dtype=np.float32)
    Wd = np.asarray(W_down, dtype=np.float32)

    in_maps = []
    pad_r = np.zeros((FP - F, D), np.float32)
    pad_c = np.zeros((D, FP - F), np.float32)
    for c in range(NCORES):
        fs = c * F
        # wg[fi, p, k, j] = Wg_pad[fi*128 + j, k*128 + p]
        wgs = np.ascontiguousarray(
            np.concatenate([Wg[fs:fs + F], pad_r], axis=0)
            .reshape(NF, P, KD, P).transpose(0, 3, 2, 1)).astype(NPBF16)
        wus = np.ascontiguousarray(
            np.concatenate([Wu[fs:fs + F], pad_r], axis=0)
            .reshape(NF, P, KD, P).transpose(0, 3, 2, 1)).astype(NPBF16)
        # wd[mi, p, k, j] = Wd_pad[mi*128 + j, fs + k*128 + p]
        wds = np.ascontiguousarray(
            np.concatenate([Wd[:, fs:fs + F], pad_c], axis=1)
            .reshape(MD, P, NF, P).transpose(0, 3, 2, 1)).astype(NPBF16)
        in_maps.append({"xh": xh, "wg": wgs, "wu": wus, "wd": wds})
    return in_maps


def _install_ntff_shim():
    """antenv.axon_hooks is missing from some images; register an
    equivalent module so trace=True can capture NTFF profiles."""
    try:
        import antenv.axon_hooks  # noqa: F401
        return True
    except ImportError:
        pass
    try:
        import antenv
        from trn_agent_boot.trn_boot import _ntff_profile_via_ctypes
        hook = _ntff_profile_via_ctypes('/opt/axon/libaxon_pjrt.so')
        mod = types.ModuleType("antenv.axon_hooks")
        mod._hook = hook
        mod.get_axon_ntff_profile_hook = lambda: mod._hook

        def set_axon_ntff_profile_hook(h):
            mod._hook = h

        mod.set_axon_ntff_profile_hook = set_axon_ntff_profile_hook
        sys.modules["antenv.axon_hooks"] = mod
        antenv.axon_hooks = mod
        return True
    except Exception:
        return False


def kernel(x, W_gate, W_up, W_down):
    global LAST_EXEC_TIME_NS, _CACHED_NC
    if _CACHED_NC is None:
        _CACHED_NC = _build()
    nc = _CACHED_NC

    in_maps = _prep_inputs(x, W_gate, W_up, W_down)

    trace = os.environ.get("MLP_KERNEL_TRACE", "0") == "1"
    if trace:
        trace = _install_ntff_shim()

    res = run_bass_kernel_spmd(nc, in_maps, list(range(NCORES)), trace=trace)
    LAST_EXEC_TIME_NS = res.exec_time_ns

    # sum per-core partials: acc[mi, p, t] = y^T[mi*128+p, t]
    acc = res.results[0]["y"].astype(np.float32, copy=True)
    for r in res.results[1:]:
        acc += r["y"]
    yout = np.ascontiguousarray(acc.transpose(2, 0, 1).reshape(T, D))
    return yout.reshape(2, 2048, D)
